# revision 48
# baseline (speedup 1.0000x reference)
"""Trainium2 Bass kernel for nn_BlockMLP (segment_reduce, memory-bound).

Computation (per batch b):
  xw[n,p,m]   = x[b,n,m,p] * w1[m]
  ys[n,p,tb]  = relu(segment_sum(xw over markers of block tb) + b1[tb])
  outs[n,p,c] = relu(sum_q ys[n,p,c*100+q] * w2[c,q] + b2[c])
  actor[b,n]  = max_p(sum_c outs*wa + ba)
  critic[b]   = mean_{n,p}(sum_c outs*wc + bc)

Distribution: data-parallel over the batch axis, one batch per NeuronCore
(B=8 = n_cores). Weights replicated.

Per-core kernel strategy:
  - Blocks (segments) are tiled 128-per-SBUF-partition-tile; each partition row
    holds one block's markers (both ploidies interleaved, as in DRAM) for all
    50 population members -> large contiguous DMA runs.
  - A custom DVE op (multiply + prefix-scan in one pass) computes running sums
    of x*w1 at 1 elem/cycle; block sums are differences of the prefix at page
    ends, so layer 1 costs a single streaming pass over the data (~80us/core)
    and the kernel stays DMA-bound (~105us/core at ~358 GB/s).
  - Layer 2 + heads contract over partitions on the TensorEngine (PSUM
    accumulation across block-tiles), then tiny vector/scalar ops finish.
"""

import os
import sys
from contextlib import ExitStack

import numpy as np

for _p in ("/opt/trn_rl_repo",):
    if os.path.isdir(_p) and _p not in sys.path:
        sys.path.insert(0, _p)

import concourse.bass as bass
import concourse.bacc as bacc
import concourse.tile as tile
from concourse import mybir
from concourse.bass import AP

F32 = mybir.dt.float32

# Problem constants (hardcoded per task contract; shapes from spec.json).
B, N, M, P = 8, 50, 93000, 2
TB, NCHR, BPC = 1000, 10, 100
ROWS = 128
NT = (TB + ROWS - 1) // ROWS  # 8 block-tiles

_MULSCAN_NAME = "MULSCAN_ANT"


def _mulscan_ref(in0, in1, c0, c1, c2):
    a = np.asarray(in0, np.float32)
    p = a.shape[0]
    a = a.reshape(p, -1)
    b = np.asarray(in1, np.float32).reshape(p, -1)
    x = a * b
    return np.cumsum(x, axis=1, dtype=np.float32).reshape(np.asarray(in0).shape)


def _register_mulscan():
    """Register the fused multiply+prefix-sum custom DVE op."""
    import concourse.dve_ops as dve_ops
    from concourse.dve_spec import AluOp, Spec, Src0, scan, lower
    from concourse.dve_uop import DveOpSpec

    if _MULSCAN_NAME in dve_ops._SUB_OPCODE_FOR_NAME:
        return next(op for op in dve_ops.OPS if op.name == _MULSCAN_NAME)

    from concourse.dve_spec import Src1

    spec = Spec(body=scan(AluOp.ADD, Src0 * Src1), reference=_mulscan_ref)
    row = max(dve_ops._SUB_OPCODE_FOR_NAME.values()) + 1
    assert row < 0x20, "custom-DVE 5-bit row field overflow"
    dve_ops._SUB_OPCODE_FOR_NAME[_MULSCAN_NAME] = row

    shas = {}
    for ver in ("v3", "v4"):
        s = DveOpSpec(
            name=_MULSCAN_NAME,
            opcode=row,
            uops=lower(spec, ver=ver),
            rd1_en=True,
        )
        shas[ver] = s.sha(ver)

    op = dve_ops.DveOp(_MULSCAN_NAME, spec, subdim=False, uops_sha=shas)
    dve_ops.OPS.append(op)
    dve_ops.CUSTOM_DVE_SPECS[_MULSCAN_NAME] = spec
    return op


MULSCAN = _register_mulscan()


# --------------------------------------------------------------------------- #
# Host-side planning from seg_ids
# --------------------------------------------------------------------------- #
class Plan:
    pass


def _build_plan(seg_ids: np.ndarray) -> Plan:
    """Derive the block tiling from (sorted) seg_ids.

    Per tile t of 128 blocks: window length L[t] (max block len in tile);
    each partition row reads a 2*L[t]-element interleaved (marker, ploidy)
    window per population member, covering its block (plus over-read that the
    zero-padded w1 arrangement cancels). Rows are grouped into DMA spans of
    constant start stride.
    """
    seg_ids = np.asarray(seg_ids).astype(np.int64)
    assert seg_ids.shape == (M,)
    lens = np.bincount(seg_ids, minlength=TB)
    assert lens.sum() == M
    starts = np.concatenate([[0], np.cumsum(lens)[:-1]])

    plan = Plan()
    plan.L = []            # window length per tile
    plan.col0 = []         # column offset of each tile in the packed w1 array
    plan.spans = []        # per tile: list of (row0, nrows, start0, stride)
    plan.rows = []         # per tile: (block_id or -1, blk_start, blk_len, wstart)

    col = 0
    for t in range(NT):
        b0 = t * ROWS
        b1 = min(b0 + ROWS, TB)
        tl = lens[b0:b1]
        L = int(max(1, tl.max())) if b1 > b0 else 1
        rows = []
        prev_ws = 0
        for r in range(ROWS):
            bid = b0 + r
            if bid < TB:
                s, ln = int(starts[bid]), int(lens[bid])
                ws = min(s, M - L)
            else:
                # pad row: re-read the previous window (w1 row is zero there)
                bid, s, ln = -1, 0, 0
                ws = prev_ws
            rows.append((bid, s, ln, ws))
            prev_ws = ws
        # group rows into constant-stride spans
        spans = []
        r = 0
        while r < ROWS:
            if r + 1 < ROWS:
                stride = rows[r + 1][3] - rows[r][3]
                r2 = r + 1
                while r2 + 1 < ROWS and rows[r2 + 1][3] - rows[r2][3] == stride:
                    r2 += 1
            else:
                stride, r2 = 0, r
            spans.append((r, r2 - r + 1, rows[r][3], stride))
            r = r2 + 1
        plan.L.append(L)
        plan.col0.append(col)
        plan.spans.append(spans)
        plan.rows.append(rows)
        col += 2 * L
    plan.w1cols = col
    return plan


def _pack_weights(plan: Plan, w1, b1, w2):
    w1p = np.zeros((ROWS, plan.w1cols), np.float32)
    b1t = np.zeros((ROWS, NT), np.float32)
    w2t = np.zeros((ROWS, NT * NCHR), np.float32)
    for t in range(NT):
        L = plan.L[t]
        c0 = plan.col0[t]
        for r, (bid, s, ln, ws) in enumerate(plan.rows[t]):
            if bid < 0 or ln == 0:
                if bid >= 0:
                    b1t[r, t] = b1[bid]
                    c = bid // BPC
                    if c < NCHR:
                        w2t[r, t * NCHR + c] = w2[c, bid % BPC]
                continue
            k0 = s - ws
            seg = w1[s : s + ln].astype(np.float32)
            cols = c0 + 2 * (k0 + np.arange(ln))
            w1p[r, cols] = seg
            w1p[r, cols + 1] = seg
            b1t[r, t] = b1[bid]
            c = bid // BPC
            if c < NCHR:
                w2t[r, t * NCHR + c] = w2[c, bid % BPC]
    return w1p, b1t, w2t


# --------------------------------------------------------------------------- #
# Bass program
# --------------------------------------------------------------------------- #
def build_program(
    plan: Plan,
    reps: int = 1,
    unroll: int = 4,
    mode: str = "full",
    dma_engine: str = "sync",
):
    do_dma = mode in ("full", "dma", "dmaflat")
    do_compute = mode in ("full", "compute")
    nc = bacc.Bacc(trn_type="TRN2")
    f32 = F32

    x_kind = "ExternalInput" if reps == 1 else "Internal"
    x_d = nc.dram_tensor("x", [N, M, P], f32, kind=x_kind)
    w1p_d = nc.dram_tensor("w1p", [ROWS, plan.w1cols], f32, kind="ExternalInput")
    b1t_d = nc.dram_tensor("b1t", [ROWS, NT], f32, kind="ExternalInput")
    w2t_d = nc.dram_tensor("w2t", [ROWS, NT * NCHR], f32, kind="ExternalInput")
    hw_d = nc.dram_tensor("hw", [NCHR, 2], f32, kind="ExternalInput")
    b2_d = nc.dram_tensor("b2", [NCHR, 1], f32, kind="ExternalInput")
    hb_d = nc.dram_tensor("hb", [1, 2], f32, kind="ExternalInput")
    actor_d = nc.dram_tensor("actor", [1, N], f32, kind="ExternalOutput")
    critic_d = nc.dram_tensor("critic", [1, 1], f32, kind="ExternalOutput")

    with ExitStack() as ctx:
        tc = ctx.enter_context(tile.TileContext(nc))
        const_pool = ctx.enter_context(tc.tile_pool(name="const", bufs=1))
        x_pool = ctx.enter_context(tc.tile_pool(name="xp", bufs=2))
        small_pool = ctx.enter_context(tc.tile_pool(name="small", bufs=2))
        out_pool = ctx.enter_context(tc.tile_pool(name="outp", bufs=1))
        psum_pool = ctx.enter_context(tc.tile_pool(name="ps", bufs=1, space="PSUM"))

        w1_sb = const_pool.tile([ROWS, plan.w1cols], f32)
        nc.sync.dma_start(w1_sb[:, :], w1p_d.ap())
        b1_sb = const_pool.tile([ROWS, NT], f32)
        nc.sync.dma_start(b1_sb[:, :], b1t_d.ap())
        w2_sb = const_pool.tile([ROWS, NT * NCHR], f32)
        nc.sync.dma_start(w2_sb[:, :], w2t_d.ap())
        hw_sb = const_pool.tile([NCHR, 2], f32)
        nc.sync.dma_start(hw_sb[:, :], hw_d.ap())
        b2_sb = const_pool.tile([NCHR, 1], f32)
        nc.sync.dma_start(b2_sb[:, :], b2_d.ap())
        hb_sb = const_pool.tile([1, 2], f32)
        nc.sync.dma_start(hb_sb[:, :], hb_d.ap())

        psum_l2 = psum_pool.tile([NCHR, 2 * N], f32)

        def tile_body(t):
            L = plan.L[t]
            F = N * 2 * L
            xt = x_pool.tile([ROWS, F], f32, tag="x")
            dma_eng = getattr(nc, dma_engine)
            if mode == "dmaflat":
                # calibration: same bytes, one flat contiguous descriptor/row
                dram = AP(x_d, 0, [[F, ROWS], [1, F]])
                dma_eng.dma_start(xt[:, :], dram)
            elif do_dma:
                for (r0, nr, start0, stride) in plan.spans[t]:
                    dram = AP(
                        x_d,
                        2 * start0,
                        [[2 * stride, nr], [2 * M, N], [1, 2 * L]],
                    )
                    dma_eng.dma_start(xt[r0 : r0 + nr, :], dram)
            if not do_compute:
                return

            x4 = xt[:, :].rearrange("q (n j t) -> q t n j", n=N, j=L, t=2)
            w3 = w1_sb[:, plan.col0[t] : plan.col0[t] + 2 * L].rearrange(
                "q (j t) -> q t j", j=L, t=2
            )
            for p in range(2):
                in0 = x4[:, p]
                in1 = w3[:, p].unsqueeze(1).broadcast_to([ROWS, N, L])
                nc.vector._custom_dve(MULSCAN, out=in0, in0=in0, in1=in1)

            # page-end prefix values; cols 0 and N+1 are zero so the diff
            # below yields per-page (block, ploidy, member) sums.
            E = small_pool.tile([ROWS, 2 * N + 2], f32, tag="E")
            nc.vector.memset(E[:, 0:1], 0.0)
            nc.vector.memset(E[:, N + 1 : N + 2], 0.0)
            ends = x4[:, :, :, L - 1]  # [128, 2, N]
            eout = AP(E.tensor, E.offset + 1, [list(E.ap[0]), [N + 1, 2], [1, N]])
            nc.scalar.activation(eout, ends, mybir.ActivationFunctionType.Copy)

            ehi = AP(E.tensor, E.offset + 1, [list(E.ap[0]), [N + 1, 2], [1, N]])
            elo = AP(E.tensor, E.offset, [list(E.ap[0]), [N + 1, 2], [1, N]])
            ys = small_pool.tile([ROWS, 2 * N], f32, tag="ys")
            ys3 = ys[:, :].rearrange("q (t n) -> q t n", t=2, n=N)
            # ys = (E_hi + b1) - E_lo
            nc.vector.scalar_tensor_tensor(
                ys3,
                ehi,
                b1_sb[:, t : t + 1],
                elo,
                mybir.AluOpType.add,
                mybir.AluOpType.subtract,
            )
            ysr = small_pool.tile([ROWS, 2 * N], f32, tag="ysr")
            nc.scalar.activation(
                ysr[:, :], ys[:, :], mybir.ActivationFunctionType.Relu
            )
            nc.tensor.matmul(
                psum_l2[:, :],
                w2_sb[:, t * NCHR : (t + 1) * NCHR],
                ysr[:, :],
                start=(t == 0),
                stop=(t == NT - 1),
            )

        if reps == 1:
            for t in range(NT):
                tile_body(t)
        else:
            assert reps % unroll == 0
            with tc.For_i(0, reps // unroll, 1) as _i:
                for _u in range(unroll):
                    for t in range(NT):
                        tile_body(t)

        outs_sb = out_pool.tile([NCHR, 2 * N], f32)
        if not do_compute:
            nc.vector.memset(outs_sb[:, :], 0.0)
        else:
            nc.scalar.activation(
                outs_sb[:, :],
                psum_l2[:, :],
                mybir.ActivationFunctionType.Relu,
                bias=b2_sb[:, :],
            )
        psum_a = psum_pool.tile([1, 2 * N], f32, tag="pa")
        nc.tensor.matmul(
            psum_a[:, :], hw_sb[:, 0:1], outs_sb[:, :], start=True, stop=True
        )
        psum_c = psum_pool.tile([1, 2 * N], f32, tag="pc")
        nc.tensor.matmul(
            psum_c[:, :], hw_sb[:, 1:2], outs_sb[:, :], start=True, stop=True
        )
        hva_sb = out_pool.tile([1, 2 * N], f32)
        nc.scalar.activation(
            hva_sb[:, :],
            psum_a[:, :],
            mybir.ActivationFunctionType.Identity,
            bias=hb_sb[:, 0:1],
        )
        hvc_sb = out_pool.tile([1, 2 * N], f32)
        nc.scalar.activation(
            hvc_sb[:, :],
            psum_c[:, :],
            mybir.ActivationFunctionType.Identity,
            bias=hb_sb[:, 1:2],
        )
        actor_sb = out_pool.tile([1, N], f32)
        nc.vector.tensor_max(actor_sb[:, :], hva_sb[0:1, 0:N], hva_sb[0:1, N : 2 * N])
        crit_sb = out_pool.tile([1, 1], f32)
        nc.vector.reduce_sum(
            crit_sb[:, :], hvc_sb[:, :], axis=mybir.AxisListType.X
        )
        crit2_sb = out_pool.tile([1, 1], f32)
        nc.vector.tensor_scalar_mul(crit2_sb[:, :], crit_sb[:, :], 1.0 / (2 * N))
        nc.sync.dma_start(actor_d.ap(), actor_sb[:, :])
        nc.sync.dma_start(critic_d.ap(), crit2_sb[:, :])

    nc.compile()
    return nc


# --------------------------------------------------------------------------- #
# v4: uniform-chromosome fast path
#
# Requires the deterministic block structure: NCHR chromosomes x BPC blocks,
# uniform block length L_c within each chromosome. Layout per tile (one
# chromosome c, one half h of the population):
#   partition p' = rrow*25 + m   (rrow: 5 groups of G=20 blocks, m: 25 members)
#   free        = the row's 20*L_c markers, both ploidies interleaved (40*L_c)
# DMA runs are 40*L_c*4 = 11-19KB -> ~2500 descriptors total (vs 51200 for the
# block-per-partition layout, which measured descriptor-bound at ~37 GB/s).
# w1 is replicated across the 25 member-slots on-chip by a PE broadcast matmul
# (stationary selector [5,125]) so the replication never touches HBM or the
# DMA fabric.
# --------------------------------------------------------------------------- #
V4_G = 20
V4_RR = BPC // V4_G  # 5 block-rows per chromosome
V4_TM = 25           # members per half
V4_PARTS = V4_RR * V4_TM  # 125


def _uniform_L(seg_ids: np.ndarray):
    """Return [L_c]*NCHR if blocks are uniform per chromosome, else None."""
    seg_ids = np.asarray(seg_ids).astype(np.int64)
    if seg_ids.shape != (M,):
        return None
    lens = np.bincount(seg_ids, minlength=TB)
    if lens.sum() != M or len(lens) != TB:
        return None
    lr = lens.reshape(NCHR, BPC)
    if not (lr == lr[:, :1]).all():
        return None
    return [int(v) for v in lr[:, 0]]


def _pack_weights_v4(Ls, w1, b1, w2, b2, wa, ba, wc, bc):
    w1 = np.asarray(w1, np.float32)
    b1 = np.asarray(b1, np.float32)
    w2 = np.asarray(w2, np.float32)
    offs = np.concatenate([[0], np.cumsum([BPC * L for L in Ls])[:-1]]).astype(int)
    W1C = sum(40 * L for L in Ls)
    w1v4 = np.zeros((V4_RR, W1C), np.float32)
    col = 0
    for c, L in enumerate(Ls):
        for rr in range(V4_RR):
            seg = w1[offs[c] + rr * V4_G * L : offs[c] + (rr + 1) * V4_G * L]
            w1v4[rr, col + 2 * np.arange(V4_G * L)] = seg
            w1v4[rr, col + 2 * np.arange(V4_G * L) + 1] = seg
        col += 40 * L
    # partition order is member-major: p' = m*5 + rr (so each member's 5 row
    # runs are consecutive in DRAM -> engines see 96KB sequential reads)
    b1g = np.zeros((V4_PARTS, NCHR * V4_G), np.float32)
    w2g = np.zeros((V4_PARTS, NCHR * V4_G), np.float32)
    for c in range(NCHR):
        for rr in range(V4_RR):
            for g in range(V4_G):
                bid = c * BPC + rr * V4_G + g
                rows = np.arange(V4_TM) * V4_RR + rr
                b1g[rows, c * V4_G + g] = b1[bid]
                w2g[rows, c * V4_G + g] = w2[c, (rr * V4_G + g)]
    b2rep = np.zeros((V4_TM, 2 * NCHR), np.float32)
    warep = np.zeros((V4_TM, 2 * NCHR), np.float32)
    wcrep = np.zeros((V4_TM, 2 * NCHR), np.float32)
    for c in range(NCHR):
        for p in range(2):
            b2rep[:, 2 * c + p] = b2[c]
            warep[:, 2 * c + p] = wa[c]
            wcrep[:, 2 * c + p] = wc[c]
    sbc = np.zeros((V4_RR, V4_PARTS), np.float32)
    for pp in range(V4_PARTS):
        sbc[pp % V4_RR, pp] = 1.0
    sum5 = np.zeros((V4_PARTS, V4_TM), np.float32)
    for pp in range(V4_PARTS):
        sum5[pp, pp // V4_RR] = 1.0
    ones25 = np.ones((V4_TM, 1), np.float32)
    barep = np.full((V4_TM, 1), float(np.asarray(ba)[0]), np.float32)
    bcv = np.array([[float(np.asarray(bc)[0])]], np.float32)
    return dict(
        w1v4=w1v4, b1g=b1g, w2g=w2g, b2rep=b2rep, warep=warep, wcrep=wcrep,
        sbc=sbc, sum5=sum5, ones25=ones25, barep=barep, bcv=bcv,
    )


def build_program_v4(Ls, reps: int = 1, unroll: int = 4, mode: str = "full"):
    do_dma = mode in ("full", "dma", "nobcast")
    do_compute = mode in ("full", "compute", "nobcast")
    nc = bacc.Bacc(trn_type="TRN2")
    f32 = F32
    PP = V4_PARTS
    W1C = sum(40 * L for L in Ls)
    offs = np.concatenate([[0], np.cumsum([BPC * L for L in Ls])[:-1]]).astype(int)

    x_kind = "ExternalInput" if reps == 1 else "Internal"
    x_d = nc.dram_tensor("x", [N, M, P], f32, kind=x_kind)
    w1v4_d = nc.dram_tensor("w1v4", [V4_RR, W1C], f32, kind="ExternalInput")
    b1g_d = nc.dram_tensor("b1g", [PP, NCHR * V4_G], f32, kind="ExternalInput")
    w2g_d = nc.dram_tensor("w2g", [PP, NCHR * V4_G], f32, kind="ExternalInput")
    b2rep_d = nc.dram_tensor("b2rep", [V4_TM, 2 * NCHR], f32, kind="ExternalInput")
    warep_d = nc.dram_tensor("warep", [V4_TM, 2 * NCHR], f32, kind="ExternalInput")
    wcrep_d = nc.dram_tensor("wcrep", [V4_TM, 2 * NCHR], f32, kind="ExternalInput")
    sbc_d = nc.dram_tensor("sbc", [V4_RR, PP], f32, kind="ExternalInput")
    sum5_d = nc.dram_tensor("sum5", [PP, V4_TM], f32, kind="ExternalInput")
    ones25_d = nc.dram_tensor("ones25", [V4_TM, 1], f32, kind="ExternalInput")
    barep_d = nc.dram_tensor("barep", [V4_TM, 1], f32, kind="ExternalInput")
    bcv_d = nc.dram_tensor("bcv", [1, 1], f32, kind="ExternalInput")
    actor_d = nc.dram_tensor("actor", [N, 1], f32, kind="ExternalOutput")
    critic_d = nc.dram_tensor("critic", [1, 1], f32, kind="ExternalOutput")

    with ExitStack() as ctx:
        tc = ctx.enter_context(tile.TileContext(nc))
        const_pool = ctx.enter_context(tc.tile_pool(name="const", bufs=1))
        x_pool = ctx.enter_context(tc.tile_pool(name="xp", bufs=4))
        w1s_pool = ctx.enter_context(tc.tile_pool(name="w1s", bufs=3))
        w1g_pool = ctx.enter_context(tc.tile_pool(name="w1g", bufs=3))
        small_pool = ctx.enter_context(tc.tile_pool(name="small", bufs=4))
        out_pool = ctx.enter_context(tc.tile_pool(name="outp", bufs=1))
        psbc_pool = ctx.enter_context(tc.tile_pool(name="pbc", bufs=3, space="PSUM"))
        psum_pool = ctx.enter_context(tc.tile_pool(name="ps", bufs=1, space="PSUM"))

        b1g_sb = const_pool.tile([PP, NCHR * V4_G], f32)
        nc.sync.dma_start(b1g_sb[:, :], b1g_d.ap())
        w2g_sb = const_pool.tile([PP, NCHR * V4_G], f32)
        nc.sync.dma_start(w2g_sb[:, :], w2g_d.ap())
        b2rep_sb = const_pool.tile([V4_TM, 2 * NCHR], f32)
        nc.sync.dma_start(b2rep_sb[:, :], b2rep_d.ap())
        warep_sb = const_pool.tile([V4_TM, 2 * NCHR], f32)
        nc.sync.dma_start(warep_sb[:, :], warep_d.ap())
        wcrep_sb = const_pool.tile([V4_TM, 2 * NCHR], f32)
        nc.sync.dma_start(wcrep_sb[:, :], wcrep_d.ap())
        sbc_sb = const_pool.tile([V4_RR, PP], f32)
        nc.sync.dma_start(sbc_sb[:, :], sbc_d.ap())
        sum5_sb = const_pool.tile([PP, V4_TM], f32)
        nc.sync.dma_start(sum5_sb[:, :], sum5_d.ap())
        ones25_sb = const_pool.tile([V4_TM, 1], f32)
        nc.sync.dma_start(ones25_sb[:, :], ones25_d.ap())
        barep_sb = const_pool.tile([V4_TM, 1], f32)
        nc.sync.dma_start(barep_sb[:, :], barep_d.ap())
        bcv_sb = const_pool.tile([1, 1], f32)
        nc.sync.dma_start(bcv_sb[:, :], bcv_d.ap())

        def body(rep):
            # all (c, h) partial sums accumulate here; one deferred matmul at
            # the end keeps the PE stream free of mid-pipeline stalls
            parts = (
                small_pool.tile([PP, 4 * NCHR], f32, tag="parts", name="parts")
                if do_compute
                else None
            )
            def bcast_w1g(c):
                """PE-broadcast chromosome c's w1 rows to all member slots."""
                L = Ls[c]
                FW = 40 * L
                colc = sum(40 * Lx for Lx in Ls[:c])
                w1g = w1g_pool.tile([PP, FW], f32, tag="w1g", name=f"w1g_{c}")
                w1s = w1s_pool.tile([V4_RR, FW], f32, tag="w1s", name=f"w1s_{c}")
                nc.sync.dma_start(
                    w1s[:, :],
                    AP(w1v4_d, colc, [[W1C, V4_RR], [1, FW]]),
                )
                for k0 in range(0, FW, 512):
                    cw = min(512, FW - k0)
                    pb = psbc_pool.tile([PP, 512], f32, tag="pbc", name=f"pbc_{c}")
                    nc.tensor.matmul(
                        pb[:, :cw],
                        sbc_sb[:, :],
                        w1s[:, k0 : k0 + cw],
                        start=True,
                        stop=True,
                    )
                    nc.scalar.activation(
                        w1g[:, k0 : k0 + cw],
                        pb[:, :cw],
                        mybir.ActivationFunctionType.Copy,
                    )
                return w1g

            w1gs = {}
            do_bcast = do_compute and mode != "nobcast"
            if do_bcast:
                w1gs[0] = bcast_w1g(0)
                w1gs[1] = bcast_w1g(1)
            for c in range(NCHR):
                L = Ls[c]
                FW = 40 * L
                if do_bcast:
                    if c + 2 < NCHR:
                        w1gs[c + 2] = bcast_w1g(c + 2)
                    w1g = w1gs.pop(c)
                else:
                    w1g = w1g_pool.tile([PP, FW], f32, tag="w1g", name=f"w1g_{c}")
                for h in range(2):
                    xt = x_pool.tile([PP, FW], f32, tag="x")
                    if do_dma:
                        # member-major order: each member's 5 runs are one
                        # DRAM-consecutive 96KB read
                        off = (h * V4_TM) * 2 * M + 2 * int(offs[c])
                        nc.sync.dma_start(
                            xt[:, :],
                            AP(x_d, off, [[2 * M, V4_TM], [1, V4_RR * FW]]),
                        )
                    if not do_compute:
                        continue
                    x3 = xt[:, :].rearrange("q (j t) -> q t j", j=V4_G * L, t=2)
                    w3 = w1g[:, :].rearrange("q (j t) -> q t j", j=V4_G * L, t=2)
                    for p in range(2):
                        nc.vector._custom_dve(
                            MULSCAN, out=x3[:, p], in0=x3[:, p], in1=w3[:, p]
                        )
                    # E cols: [0]=0, [1..G]=p0 ends, [G+1]=0, [G+2..2G+1]=p1 ends
                    E = small_pool.tile([PP, 2 * V4_G + 2], f32, tag="E")
                    nc.vector.memset(E[:, 0:1], 0.0)
                    nc.vector.memset(E[:, V4_G + 1 : V4_G + 2], 0.0)
                    eins = AP(
                        xt.tensor,
                        xt.offset + 2 * L - 2,
                        [list(xt.ap[0]), [1, 2], [2 * L, V4_G]],
                    )
                    eout = AP(
                        E.tensor,
                        E.offset + 1,
                        [list(E.ap[0]), [V4_G + 1, 2], [1, V4_G]],
                    )
                    nc.scalar.activation(
                        eout, eins, mybir.ActivationFunctionType.Copy
                    )
                    ehi = AP(
                        E.tensor, E.offset + 1,
                        [list(E.ap[0]), [V4_G + 1, 2], [1, V4_G]],
                    )
                    elo = AP(
                        E.tensor, E.offset,
                        [list(E.ap[0]), [V4_G + 1, 2], [1, V4_G]],
                    )
                    ys = small_pool.tile([PP, 2 * V4_G], f32, tag="ys")
                    ys3 = AP(
                        ys.tensor, ys.offset,
                        [list(ys.ap[0]), [V4_G, 2], [1, V4_G]],
                    )
                    nc.vector.tensor_sub(ys3, ehi, elo)
                    b1b = AP(
                        b1g_sb.tensor,
                        b1g_sb.offset + c * V4_G,
                        [list(b1g_sb.ap[0]), [0, 2], [1, V4_G]],
                    )
                    nc.vector.tensor_add(ys3, ys3, b1b)
                    ysr = small_pool.tile([PP, 2 * V4_G], f32, tag="ysr")
                    nc.scalar.activation(
                        ysr[:, :], ys[:, : 2 * V4_G],
                        mybir.ActivationFunctionType.Relu,
                    )
                    yw = small_pool.tile([PP, 2 * V4_G], f32, tag="yw")
                    w2b = AP(
                        w2g_sb.tensor,
                        w2g_sb.offset + c * V4_G,
                        [list(w2g_sb.ap[0]), [0, 2], [1, V4_G]],
                    )
                    nc.vector.tensor_mul(yw[:, :], ysr[:, :], w2b)
                    yw3 = AP(
                        yw.tensor, yw.offset,
                        [list(yw.ap[0]), [V4_G, 2], [1, V4_G]],
                    )
                    pcol = h * 2 * NCHR + 2 * c
                    nc.vector.reduce_sum(
                        parts[:, pcol : pcol + 2], yw3, axis=mybir.AxisListType.X
                    )
            if not do_compute:
                return
            psum_o = psum_pool.tile(
                [V4_TM, 4 * NCHR], f32, tag=f"po_{rep % 2}", name="psum_o"
            )
            nc.tensor.matmul(
                psum_o[:, :], sum5_sb[:, :], parts[:, :], start=True, stop=True
            )
            pcrit = psum_pool.tile([1, 1], f32, tag=f"pc_{rep % 2}")
            for h in range(2):
                outs = small_pool.tile([V4_TM, 2 * NCHR], f32, tag="outs")
                nc.vector.tensor_add(
                    outs[:, :],
                    psum_o[:, h * 2 * NCHR : (h + 1) * 2 * NCHR],
                    b2rep_sb[:, :],
                )
                outsr = small_pool.tile([V4_TM, 2 * NCHR], f32, tag="outsr")
                nc.scalar.activation(
                    outsr[:, :], outs[:, :], mybir.ActivationFunctionType.Relu
                )
                # actor: sum_c outs*wa (per p), max over p, + ba
                aw = small_pool.tile([V4_TM, 2 * NCHR], f32, tag="aw")
                nc.vector.tensor_mul(aw[:, :], outsr[:, :], warep_sb[:, :])
                av = small_pool.tile([V4_TM, 2], f32, tag="av")
                aw3 = AP(aw.tensor, aw.offset, [list(aw.ap[0]), [1, 2], [2, NCHR]])
                nc.vector.reduce_sum(av[:, :], aw3, axis=mybir.AxisListType.X)
                amax = small_pool.tile([V4_TM, 1], f32, tag="amax")
                nc.vector.tensor_max(amax[:, :], av[:, 0:1], av[:, 1:2])
                actor_h = small_pool.tile([V4_TM, 1], f32, tag="actorh")
                nc.scalar.activation(
                    actor_h[:, :], amax[:, :],
                    mybir.ActivationFunctionType.Identity, bias=barep_sb[:, :],
                )
                nc.sync.dma_start(
                    actor_d.ap()[h * V4_TM : (h + 1) * V4_TM, :], actor_h[:, :]
                )
                # critic partial: sum_{p,c} outs*wc
                cw_ = small_pool.tile([V4_TM, 2 * NCHR], f32, tag="cw")
                nc.vector.tensor_mul(cw_[:, :], outsr[:, :], wcrep_sb[:, :])
                cv = small_pool.tile([V4_TM, 1], f32, tag="cv")
                nc.vector.reduce_sum(
                    cv[:, :], cw_[:, :], axis=mybir.AxisListType.X
                )
                nc.tensor.matmul(
                    pcrit[:, :], ones25_sb[:, :], cv[:, :],
                    start=(h == 0), stop=(h == 1),
                )
            crit_sb = out_pool.tile([1, 1], f32, tag="crit")
            nc.scalar.activation(
                crit_sb[:, :], pcrit[:, :],
                mybir.ActivationFunctionType.Identity,
                bias=bcv_sb[:, :], scale=1.0 / (2 * N),
            )
            nc.sync.dma_start(critic_d.ap(), crit_sb[:, :])

        if reps == 1:
            body(0)
        else:
            assert reps % unroll == 0
            with tc.For_i(0, reps // unroll, 1) as _i:
                for u in range(unroll):
                    body(u)

    nc.compile()
    return nc


# --------------------------------------------------------------------------- #
# Entry point
# --------------------------------------------------------------------------- #
_CACHE = {}


def _get_program(seg_key, seg_ids):
    if seg_key not in _CACHE:
        Ls = _uniform_L(seg_ids)
        if Ls is not None:
            nc = build_program_v4(Ls)
            _CACHE[seg_key] = (("v4", Ls), nc)
        else:
            plan = _build_plan(seg_ids)
            nc = build_program(plan)
            _CACHE[seg_key] = (("v1", plan), nc)
    return _CACHE[seg_key]


def make_in_maps_v4(Ls, x, w1, b1, w2, b2, wa, ba, wc, bc):
    packed = _pack_weights_v4(Ls, w1, b1, w2, b2, wa, ba, wc, bc)
    x = np.asarray(x)
    maps = []
    for i in range(x.shape[0]):
        m = {"x": np.ascontiguousarray(x[i], np.float32)}
        m.update(packed)
        maps.append(m)
    return maps


def make_in_maps(plan, x, w1, b1, w2, b2, wa, ba, wc, bc):
    w1p, b1t, w2t = _pack_weights(plan, np.asarray(w1), np.asarray(b1), np.asarray(w2))
    hw = np.stack([np.asarray(wa), np.asarray(wc)], axis=1).astype(np.float32)
    b2c = np.asarray(b2, np.float32).reshape(NCHR, 1)
    hb = np.array([[float(np.asarray(ba)[0]), float(np.asarray(bc)[0])]], np.float32)
    x = np.asarray(x)
    maps = []
    for i in range(x.shape[0]):
        maps.append(
            {
                "x": np.ascontiguousarray(x[i], np.float32),
                "w1p": w1p,
                "b1t": b1t,
                "w2t": w2t,
                "hw": hw,
                "b2": b2c,
                "hb": hb,
            }
        )
    return maps


def kernel(x, w1, b1, w2, b2, wa, ba, wc, bc, seg_ids):
    from concourse import bass_utils

    seg_np = np.asarray(seg_ids)
    seg_key = hash(seg_np.tobytes())
    (kind, meta), nc = _get_program(seg_key, seg_np)
    if kind == "v4":
        in_maps = make_in_maps_v4(meta, x, w1, b1, w2, b2, wa, ba, wc, bc)
    else:
        in_maps = make_in_maps(meta, x, w1, b1, w2, b2, wa, ba, wc, bc)
    res = bass_utils.run_bass_kernel_spmd(nc, in_maps, core_ids=list(range(B)))
    actor = np.stack(
        [np.asarray(res.results[i]["actor"]).reshape(N) for i in range(B)]
    ).astype(np.float32)
    critic = np.array(
        [float(np.asarray(res.results[i]["critic"]).reshape(())) for i in range(B)],
        np.float32,
    )
    return actor, critic


# revision 51
# speedup vs baseline: 1.1989x; 1.1989x over previous
"""Trainium2 Bass kernel for nn_BlockMLP (segment_reduce, memory-bound).

Computation (per batch b):
  xw[n,p,m]   = x[b,n,m,p] * w1[m]
  ys[n,p,tb]  = relu(segment_sum(xw over markers of block tb) + b1[tb])
  outs[n,p,c] = relu(sum_q ys[n,p,c*100+q] * w2[c,q] + b2[c])
  actor[b,n]  = max_p(sum_c outs*wa + ba)
  critic[b]   = mean_{n,p}(sum_c outs*wc + bc)

Distribution: data-parallel over the batch axis, one batch per NeuronCore
(B=8 = n_cores). Weights replicated.

Per-core kernel strategy (v4 fast path, used for the deterministic seg_ids
structure of 10 chromosomes x 100 uniform-length blocks):
  - One tile per (chromosome, population-half): 125 partitions = 25 members x
    5 block-rows of 20 blocks each, member-major so each member's chromosome
    span is a single 96KB DRAM-sequential read (~2.5K descriptors total).
  - A custom DVE op (multiply + prefix-scan fused in one pass, 1 elem/cycle)
    computes running sums of x*w1 in place; per-block sums are differences of
    the prefix at block boundaries (uniform stride within a chromosome tile).
  - w1 is replicated to the 25 member slots on-chip by a PE broadcast matmul
    (pipelined two chromosomes ahead), so replication never touches HBM.
  - Layer 2 is a per-row weighted reduce (w2) + one deferred "sum the 5
    block-rows" matmul on the PE; heads finish with tiny vector/scalar ops.

A generic fallback path (block-per-partition window tiling, any sorted
seg_ids) handles non-uniform segment structures.
"""

import os
import sys
from contextlib import ExitStack

import numpy as np

for _p in ("/opt/trn_rl_repo",):
    if os.path.isdir(_p) and _p not in sys.path:
        sys.path.insert(0, _p)

import concourse.bass as bass
import concourse.bacc as bacc
import concourse.tile as tile
from concourse import mybir
from concourse.bass import AP

F32 = mybir.dt.float32

# Problem constants (hardcoded per task contract; shapes from spec.json).
B, N, M, P = 8, 50, 93000, 2
TB, NCHR, BPC = 1000, 10, 100
ROWS = 128
NT = (TB + ROWS - 1) // ROWS  # 8 block-tiles

_MULSCAN_NAME = "MULSCAN_ANT"


def _mulscan_ref(in0, in1, c0, c1, c2):
    a = np.asarray(in0, np.float32)
    p = a.shape[0]
    a = a.reshape(p, -1)
    b = np.asarray(in1, np.float32).reshape(p, -1)
    x = a * b
    return np.cumsum(x, axis=1, dtype=np.float32).reshape(np.asarray(in0).shape)


def _register_mulscan():
    """Register the fused multiply+prefix-sum custom DVE op."""
    import concourse.dve_ops as dve_ops
    from concourse.dve_spec import AluOp, Spec, Src0, scan, lower
    from concourse.dve_uop import DveOpSpec

    if _MULSCAN_NAME in dve_ops._SUB_OPCODE_FOR_NAME:
        return next(op for op in dve_ops.OPS if op.name == _MULSCAN_NAME)

    from concourse.dve_spec import Src1

    spec = Spec(body=scan(AluOp.ADD, Src0 * Src1), reference=_mulscan_ref)
    row = max(dve_ops._SUB_OPCODE_FOR_NAME.values()) + 1
    assert row < 0x20, "custom-DVE 5-bit row field overflow"
    dve_ops._SUB_OPCODE_FOR_NAME[_MULSCAN_NAME] = row

    shas = {}
    for ver in ("v3", "v4"):
        s = DveOpSpec(
            name=_MULSCAN_NAME,
            opcode=row,
            uops=lower(spec, ver=ver),
            rd1_en=True,
        )
        shas[ver] = s.sha(ver)

    op = dve_ops.DveOp(_MULSCAN_NAME, spec, subdim=False, uops_sha=shas)
    dve_ops.OPS.append(op)
    dve_ops.CUSTOM_DVE_SPECS[_MULSCAN_NAME] = spec
    return op


MULSCAN = _register_mulscan()


# --------------------------------------------------------------------------- #
# Host-side planning from seg_ids
# --------------------------------------------------------------------------- #
class Plan:
    pass


def _build_plan(seg_ids: np.ndarray) -> Plan:
    """Derive the block tiling from (sorted) seg_ids.

    Per tile t of 128 blocks: window length L[t] (max block len in tile);
    each partition row reads a 2*L[t]-element interleaved (marker, ploidy)
    window per population member, covering its block (plus over-read that the
    zero-padded w1 arrangement cancels). Rows are grouped into DMA spans of
    constant start stride.
    """
    seg_ids = np.asarray(seg_ids).astype(np.int64)
    assert seg_ids.shape == (M,)
    lens = np.bincount(seg_ids, minlength=TB)
    assert lens.sum() == M
    starts = np.concatenate([[0], np.cumsum(lens)[:-1]])

    plan = Plan()
    plan.L = []            # window length per tile
    plan.col0 = []         # column offset of each tile in the packed w1 array
    plan.spans = []        # per tile: list of (row0, nrows, start0, stride)
    plan.rows = []         # per tile: (block_id or -1, blk_start, blk_len, wstart)

    col = 0
    for t in range(NT):
        b0 = t * ROWS
        b1 = min(b0 + ROWS, TB)
        tl = lens[b0:b1]
        L = int(max(1, tl.max())) if b1 > b0 else 1
        rows = []
        prev_ws = 0
        for r in range(ROWS):
            bid = b0 + r
            if bid < TB:
                s, ln = int(starts[bid]), int(lens[bid])
                ws = min(s, M - L)
            else:
                # pad row: re-read the previous window (w1 row is zero there)
                bid, s, ln = -1, 0, 0
                ws = prev_ws
            rows.append((bid, s, ln, ws))
            prev_ws = ws
        # group rows into constant-stride spans
        spans = []
        r = 0
        while r < ROWS:
            if r + 1 < ROWS:
                stride = rows[r + 1][3] - rows[r][3]
                r2 = r + 1
                while r2 + 1 < ROWS and rows[r2 + 1][3] - rows[r2][3] == stride:
                    r2 += 1
            else:
                stride, r2 = 0, r
            spans.append((r, r2 - r + 1, rows[r][3], stride))
            r = r2 + 1
        plan.L.append(L)
        plan.col0.append(col)
        plan.spans.append(spans)
        plan.rows.append(rows)
        col += 2 * L
    plan.w1cols = col
    return plan


def _pack_weights(plan: Plan, w1, b1, w2):
    w1p = np.zeros((ROWS, plan.w1cols), np.float32)
    b1t = np.zeros((ROWS, NT), np.float32)
    w2t = np.zeros((ROWS, NT * NCHR), np.float32)
    for t in range(NT):
        L = plan.L[t]
        c0 = plan.col0[t]
        for r, (bid, s, ln, ws) in enumerate(plan.rows[t]):
            if bid < 0 or ln == 0:
                if bid >= 0:
                    b1t[r, t] = b1[bid]
                    c = bid // BPC
                    if c < NCHR:
                        w2t[r, t * NCHR + c] = w2[c, bid % BPC]
                continue
            k0 = s - ws
            seg = w1[s : s + ln].astype(np.float32)
            cols = c0 + 2 * (k0 + np.arange(ln))
            w1p[r, cols] = seg
            w1p[r, cols + 1] = seg
            b1t[r, t] = b1[bid]
            c = bid // BPC
            if c < NCHR:
                w2t[r, t * NCHR + c] = w2[c, bid % BPC]
    return w1p, b1t, w2t


# --------------------------------------------------------------------------- #
# Bass program
# --------------------------------------------------------------------------- #
def build_program(
    plan: Plan,
    reps: int = 1,
    unroll: int = 4,
    mode: str = "full",
    dma_engine: str = "sync",
):
    do_dma = mode in ("full", "dma", "dmaflat")
    do_compute = mode in ("full", "compute")
    nc = bacc.Bacc(trn_type="TRN2")
    f32 = F32

    x_kind = "ExternalInput" if reps == 1 else "Internal"
    x_d = nc.dram_tensor("x", [N, M, P], f32, kind=x_kind)
    w1p_d = nc.dram_tensor("w1p", [ROWS, plan.w1cols], f32, kind="ExternalInput")
    b1t_d = nc.dram_tensor("b1t", [ROWS, NT], f32, kind="ExternalInput")
    w2t_d = nc.dram_tensor("w2t", [ROWS, NT * NCHR], f32, kind="ExternalInput")
    hw_d = nc.dram_tensor("hw", [NCHR, 2], f32, kind="ExternalInput")
    b2_d = nc.dram_tensor("b2", [NCHR, 1], f32, kind="ExternalInput")
    hb_d = nc.dram_tensor("hb", [1, 2], f32, kind="ExternalInput")
    actor_d = nc.dram_tensor("actor", [1, N], f32, kind="ExternalOutput")
    critic_d = nc.dram_tensor("critic", [1, 1], f32, kind="ExternalOutput")

    with ExitStack() as ctx:
        tc = ctx.enter_context(tile.TileContext(nc))
        const_pool = ctx.enter_context(tc.tile_pool(name="const", bufs=1))
        x_pool = ctx.enter_context(tc.tile_pool(name="xp", bufs=2))
        small_pool = ctx.enter_context(tc.tile_pool(name="small", bufs=2))
        out_pool = ctx.enter_context(tc.tile_pool(name="outp", bufs=1))
        psum_pool = ctx.enter_context(tc.tile_pool(name="ps", bufs=1, space="PSUM"))

        w1_sb = const_pool.tile([ROWS, plan.w1cols], f32)
        nc.sync.dma_start(w1_sb[:, :], w1p_d.ap())
        b1_sb = const_pool.tile([ROWS, NT], f32)
        nc.sync.dma_start(b1_sb[:, :], b1t_d.ap())
        w2_sb = const_pool.tile([ROWS, NT * NCHR], f32)
        nc.sync.dma_start(w2_sb[:, :], w2t_d.ap())
        hw_sb = const_pool.tile([NCHR, 2], f32)
        nc.sync.dma_start(hw_sb[:, :], hw_d.ap())
        b2_sb = const_pool.tile([NCHR, 1], f32)
        nc.sync.dma_start(b2_sb[:, :], b2_d.ap())
        hb_sb = const_pool.tile([1, 2], f32)
        nc.sync.dma_start(hb_sb[:, :], hb_d.ap())

        psum_l2 = psum_pool.tile([NCHR, 2 * N], f32)

        def tile_body(t):
            L = plan.L[t]
            F = N * 2 * L
            xt = x_pool.tile([ROWS, F], f32, tag="x")
            dma_eng = getattr(nc, dma_engine)
            if mode == "dmaflat":
                # calibration: same bytes, one flat contiguous descriptor/row
                dram = AP(x_d, 0, [[F, ROWS], [1, F]])
                dma_eng.dma_start(xt[:, :], dram)
            elif do_dma:
                for (r0, nr, start0, stride) in plan.spans[t]:
                    dram = AP(
                        x_d,
                        2 * start0,
                        [[2 * stride, nr], [2 * M, N], [1, 2 * L]],
                    )
                    dma_eng.dma_start(xt[r0 : r0 + nr, :], dram)
            if not do_compute:
                return

            x4 = xt[:, :].rearrange("q (n j t) -> q t n j", n=N, j=L, t=2)
            w3 = w1_sb[:, plan.col0[t] : plan.col0[t] + 2 * L].rearrange(
                "q (j t) -> q t j", j=L, t=2
            )
            for p in range(2):
                in0 = x4[:, p]
                in1 = w3[:, p].unsqueeze(1).broadcast_to([ROWS, N, L])
                nc.vector._custom_dve(MULSCAN, out=in0, in0=in0, in1=in1)

            # page-end prefix values; cols 0 and N+1 are zero so the diff
            # below yields per-page (block, ploidy, member) sums.
            E = small_pool.tile([ROWS, 2 * N + 2], f32, tag="E")
            nc.vector.memset(E[:, 0:1], 0.0)
            nc.vector.memset(E[:, N + 1 : N + 2], 0.0)
            ends = x4[:, :, :, L - 1]  # [128, 2, N]
            eout = AP(E.tensor, E.offset + 1, [list(E.ap[0]), [N + 1, 2], [1, N]])
            nc.scalar.activation(eout, ends, mybir.ActivationFunctionType.Copy)

            ehi = AP(E.tensor, E.offset + 1, [list(E.ap[0]), [N + 1, 2], [1, N]])
            elo = AP(E.tensor, E.offset, [list(E.ap[0]), [N + 1, 2], [1, N]])
            ys = small_pool.tile([ROWS, 2 * N], f32, tag="ys")
            ys3 = ys[:, :].rearrange("q (t n) -> q t n", t=2, n=N)
            # ys = (E_hi + b1) - E_lo
            nc.vector.scalar_tensor_tensor(
                ys3,
                ehi,
                b1_sb[:, t : t + 1],
                elo,
                mybir.AluOpType.add,
                mybir.AluOpType.subtract,
            )
            ysr = small_pool.tile([ROWS, 2 * N], f32, tag="ysr")
            nc.scalar.activation(
                ysr[:, :], ys[:, :], mybir.ActivationFunctionType.Relu
            )
            nc.tensor.matmul(
                psum_l2[:, :],
                w2_sb[:, t * NCHR : (t + 1) * NCHR],
                ysr[:, :],
                start=(t == 0),
                stop=(t == NT - 1),
            )

        if reps == 1:
            for t in range(NT):
                tile_body(t)
        else:
            assert reps % unroll == 0
            with tc.For_i(0, reps // unroll, 1) as _i:
                for _u in range(unroll):
                    for t in range(NT):
                        tile_body(t)

        outs_sb = out_pool.tile([NCHR, 2 * N], f32)
        if not do_compute:
            nc.vector.memset(outs_sb[:, :], 0.0)
        else:
            nc.scalar.activation(
                outs_sb[:, :],
                psum_l2[:, :],
                mybir.ActivationFunctionType.Relu,
                bias=b2_sb[:, :],
            )
        psum_a = psum_pool.tile([1, 2 * N], f32, tag="pa")
        nc.tensor.matmul(
            psum_a[:, :], hw_sb[:, 0:1], outs_sb[:, :], start=True, stop=True
        )
        psum_c = psum_pool.tile([1, 2 * N], f32, tag="pc")
        nc.tensor.matmul(
            psum_c[:, :], hw_sb[:, 1:2], outs_sb[:, :], start=True, stop=True
        )
        hva_sb = out_pool.tile([1, 2 * N], f32)
        nc.scalar.activation(
            hva_sb[:, :],
            psum_a[:, :],
            mybir.ActivationFunctionType.Identity,
            bias=hb_sb[:, 0:1],
        )
        hvc_sb = out_pool.tile([1, 2 * N], f32)
        nc.scalar.activation(
            hvc_sb[:, :],
            psum_c[:, :],
            mybir.ActivationFunctionType.Identity,
            bias=hb_sb[:, 1:2],
        )
        actor_sb = out_pool.tile([1, N], f32)
        nc.vector.tensor_max(actor_sb[:, :], hva_sb[0:1, 0:N], hva_sb[0:1, N : 2 * N])
        crit_sb = out_pool.tile([1, 1], f32)
        nc.vector.reduce_sum(
            crit_sb[:, :], hvc_sb[:, :], axis=mybir.AxisListType.X
        )
        crit2_sb = out_pool.tile([1, 1], f32)
        nc.vector.tensor_scalar_mul(crit2_sb[:, :], crit_sb[:, :], 1.0 / (2 * N))
        nc.sync.dma_start(actor_d.ap(), actor_sb[:, :])
        nc.sync.dma_start(critic_d.ap(), crit2_sb[:, :])

    nc.compile()
    return nc


# --------------------------------------------------------------------------- #
# v4: uniform-chromosome fast path
#
# Requires the deterministic block structure: NCHR chromosomes x BPC blocks,
# uniform block length L_c within each chromosome. Layout per tile (one
# chromosome c, one half h of the population):
#   partition p' = rrow*25 + m   (rrow: 5 groups of G=20 blocks, m: 25 members)
#   free        = the row's 20*L_c markers, both ploidies interleaved (40*L_c)
# DMA runs are 40*L_c*4 = 11-19KB -> ~2500 descriptors total (vs 51200 for the
# block-per-partition layout, which measured descriptor-bound at ~37 GB/s).
# w1 is replicated across the 25 member-slots on-chip by a PE broadcast matmul
# (stationary selector [5,125]) so the replication never touches HBM or the
# DMA fabric.
# --------------------------------------------------------------------------- #
V4_G = 20
V4_RR = BPC // V4_G  # 5 block-rows per chromosome
V4_TM = 25           # members per half
V4_PARTS = V4_RR * V4_TM  # 125


def _uniform_L(seg_ids: np.ndarray):
    """Return [L_c]*NCHR if blocks are uniform per chromosome, else None."""
    seg_ids = np.asarray(seg_ids).astype(np.int64)
    if seg_ids.shape != (M,):
        return None
    lens = np.bincount(seg_ids, minlength=TB)
    if lens.sum() != M or len(lens) != TB:
        return None
    lr = lens.reshape(NCHR, BPC)
    if not (lr == lr[:, :1]).all():
        return None
    return [int(v) for v in lr[:, 0]]


def _pack_weights_v4(Ls, w1, b1, w2, b2, wa, ba, wc, bc):
    w1 = np.asarray(w1, np.float32)
    b1 = np.asarray(b1, np.float32)
    w2 = np.asarray(w2, np.float32)
    offs = np.concatenate([[0], np.cumsum([BPC * L for L in Ls])[:-1]]).astype(int)
    W1C = sum(40 * L for L in Ls)
    w1v4 = np.zeros((V4_RR, W1C), np.float32)
    col = 0
    for c, L in enumerate(Ls):
        for rr in range(V4_RR):
            seg = w1[offs[c] + rr * V4_G * L : offs[c] + (rr + 1) * V4_G * L]
            w1v4[rr, col + 2 * np.arange(V4_G * L)] = seg
            w1v4[rr, col + 2 * np.arange(V4_G * L) + 1] = seg
        col += 40 * L
    # partition order is member-major: p' = m*5 + rr (so each member's 5 row
    # runs are consecutive in DRAM -> engines see 96KB sequential reads)
    b1g = np.zeros((V4_PARTS, NCHR * V4_G), np.float32)
    w2g = np.zeros((V4_PARTS, NCHR * V4_G), np.float32)
    for c in range(NCHR):
        for rr in range(V4_RR):
            for g in range(V4_G):
                bid = c * BPC + rr * V4_G + g
                rows = np.arange(V4_TM) * V4_RR + rr
                b1g[rows, c * V4_G + g] = b1[bid]
                w2g[rows, c * V4_G + g] = w2[c, (rr * V4_G + g)]
    b2rep = np.zeros((V4_TM, 2 * NCHR), np.float32)
    warep = np.zeros((V4_TM, 2 * NCHR), np.float32)
    wcrep = np.zeros((V4_TM, 2 * NCHR), np.float32)
    for c in range(NCHR):
        for p in range(2):
            b2rep[:, 2 * c + p] = b2[c]
            warep[:, 2 * c + p] = wa[c]
            wcrep[:, 2 * c + p] = wc[c]
    sbc = np.zeros((V4_RR, V4_PARTS), np.float32)
    for pp in range(V4_PARTS):
        sbc[pp % V4_RR, pp] = 1.0
    sum5 = np.zeros((V4_PARTS, V4_TM), np.float32)
    for pp in range(V4_PARTS):
        sum5[pp, pp // V4_RR] = 1.0
    ones25 = np.ones((V4_TM, 1), np.float32)
    barep = np.full((V4_TM, 1), float(np.asarray(ba)[0]), np.float32)
    bcv = np.array([[float(np.asarray(bc)[0])]], np.float32)
    return dict(
        w1v4=w1v4, b1g=b1g, w2g=w2g, b2rep=b2rep, warep=warep, wcrep=wcrep,
        sbc=sbc, sum5=sum5, ones25=ones25, barep=barep, bcv=bcv,
    )


def build_program_v4(Ls, reps: int = 1, unroll: int = 4, mode: str = "full"):
    do_dma = mode in ("full", "dma", "nobcast")
    do_compute = mode in ("full", "compute", "nobcast")
    nc = bacc.Bacc(trn_type="TRN2")
    f32 = F32
    PP = V4_PARTS
    W1C = sum(40 * L for L in Ls)
    offs = np.concatenate([[0], np.cumsum([BPC * L for L in Ls])[:-1]]).astype(int)

    x_kind = "ExternalInput" if reps == 1 else "Internal"
    x_d = nc.dram_tensor("x", [N, M, P], f32, kind=x_kind)
    w1v4_d = nc.dram_tensor("w1v4", [V4_RR, W1C], f32, kind="ExternalInput")
    b1g_d = nc.dram_tensor("b1g", [PP, NCHR * V4_G], f32, kind="ExternalInput")
    w2g_d = nc.dram_tensor("w2g", [PP, NCHR * V4_G], f32, kind="ExternalInput")
    b2rep_d = nc.dram_tensor("b2rep", [V4_TM, 2 * NCHR], f32, kind="ExternalInput")
    warep_d = nc.dram_tensor("warep", [V4_TM, 2 * NCHR], f32, kind="ExternalInput")
    wcrep_d = nc.dram_tensor("wcrep", [V4_TM, 2 * NCHR], f32, kind="ExternalInput")
    sbc_d = nc.dram_tensor("sbc", [V4_RR, PP], f32, kind="ExternalInput")
    sum5_d = nc.dram_tensor("sum5", [PP, V4_TM], f32, kind="ExternalInput")
    ones25_d = nc.dram_tensor("ones25", [V4_TM, 1], f32, kind="ExternalInput")
    barep_d = nc.dram_tensor("barep", [V4_TM, 1], f32, kind="ExternalInput")
    bcv_d = nc.dram_tensor("bcv", [1, 1], f32, kind="ExternalInput")
    actor_d = nc.dram_tensor("actor", [N, 1], f32, kind="ExternalOutput")
    critic_d = nc.dram_tensor("critic", [1, 1], f32, kind="ExternalOutput")

    with ExitStack() as ctx:
        tc = ctx.enter_context(tile.TileContext(nc))
        const_pool = ctx.enter_context(tc.tile_pool(name="const", bufs=1))
        x_pool = ctx.enter_context(tc.tile_pool(name="xp", bufs=4))
        w1s_pool = ctx.enter_context(tc.tile_pool(name="w1s", bufs=3))
        w1g_pool = ctx.enter_context(tc.tile_pool(name="w1g", bufs=3))
        small_pool = ctx.enter_context(tc.tile_pool(name="small", bufs=4))
        out_pool = ctx.enter_context(tc.tile_pool(name="outp", bufs=1))
        psbc_pool = ctx.enter_context(tc.tile_pool(name="pbc", bufs=3, space="PSUM"))
        psum_pool = ctx.enter_context(tc.tile_pool(name="ps", bufs=1, space="PSUM"))

        b1g_sb = const_pool.tile([PP, NCHR * V4_G], f32)
        nc.sync.dma_start(b1g_sb[:, :], b1g_d.ap())
        w2g_sb = const_pool.tile([PP, NCHR * V4_G], f32)
        nc.sync.dma_start(w2g_sb[:, :], w2g_d.ap())
        b2rep_sb = const_pool.tile([V4_TM, 2 * NCHR], f32)
        nc.sync.dma_start(b2rep_sb[:, :], b2rep_d.ap())
        warep_sb = const_pool.tile([V4_TM, 2 * NCHR], f32)
        nc.sync.dma_start(warep_sb[:, :], warep_d.ap())
        wcrep_sb = const_pool.tile([V4_TM, 2 * NCHR], f32)
        nc.sync.dma_start(wcrep_sb[:, :], wcrep_d.ap())
        sbc_sb = const_pool.tile([V4_RR, PP], f32)
        nc.sync.dma_start(sbc_sb[:, :], sbc_d.ap())
        sum5_sb = const_pool.tile([PP, V4_TM], f32)
        nc.sync.dma_start(sum5_sb[:, :], sum5_d.ap())
        ones25_sb = const_pool.tile([V4_TM, 1], f32)
        nc.sync.dma_start(ones25_sb[:, :], ones25_d.ap())
        barep_sb = const_pool.tile([V4_TM, 1], f32)
        nc.sync.dma_start(barep_sb[:, :], barep_d.ap())
        bcv_sb = const_pool.tile([1, 1], f32)
        nc.sync.dma_start(bcv_sb[:, :], bcv_d.ap())

        def body(rep):
            # all (c, h) partial sums accumulate here; one deferred matmul at
            # the end keeps the PE stream free of mid-pipeline stalls
            parts = (
                small_pool.tile([PP, 4 * NCHR], f32, tag="parts", name="parts")
                if do_compute
                else None
            )
            def bcast_w1g(c):
                """PE-broadcast chromosome c's w1 rows to all member slots."""
                L = Ls[c]
                FW = 40 * L
                colc = sum(40 * Lx for Lx in Ls[:c])
                w1g = w1g_pool.tile([PP, FW], f32, tag="w1g", name=f"w1g_{c}")
                w1s = w1s_pool.tile([V4_RR, FW], f32, tag="w1s", name=f"w1s_{c}")
                nc.sync.dma_start(
                    w1s[:, :],
                    AP(w1v4_d, colc, [[W1C, V4_RR], [1, FW]]),
                )
                for k0 in range(0, FW, 512):
                    cw = min(512, FW - k0)
                    pb = psbc_pool.tile([PP, 512], f32, tag="pbc", name=f"pbc_{c}")
                    nc.tensor.matmul(
                        pb[:, :cw],
                        sbc_sb[:, :],
                        w1s[:, k0 : k0 + cw],
                        start=True,
                        stop=True,
                    )
                    nc.scalar.activation(
                        w1g[:, k0 : k0 + cw],
                        pb[:, :cw],
                        mybir.ActivationFunctionType.Copy,
                    )
                return w1g

            w1gs = {}
            do_bcast = do_compute and mode != "nobcast"
            if do_bcast:
                w1gs[0] = bcast_w1g(0)
                w1gs[1] = bcast_w1g(1)
            for c in range(NCHR):
                L = Ls[c]
                FW = 40 * L
                if do_bcast:
                    if c + 2 < NCHR:
                        w1gs[c + 2] = bcast_w1g(c + 2)
                    w1g = w1gs.pop(c)
                else:
                    w1g = w1g_pool.tile([PP, FW], f32, tag="w1g", name=f"w1g_{c}")
                for h in range(2):
                    xt = x_pool.tile([PP, FW], f32, tag="x")
                    if do_dma:
                        # member-major order: each member's 5 runs are one
                        # DRAM-consecutive 96KB read
                        off = (h * V4_TM) * 2 * M + 2 * int(offs[c])
                        nc.sync.dma_start(
                            xt[:, :],
                            AP(x_d, off, [[2 * M, V4_TM], [1, V4_RR * FW]]),
                        )
                    if not do_compute:
                        continue
                    x3 = xt[:, :].rearrange("q (j t) -> q t j", j=V4_G * L, t=2)
                    w3 = w1g[:, :].rearrange("q (j t) -> q t j", j=V4_G * L, t=2)
                    for p in range(2):
                        nc.vector._custom_dve(
                            MULSCAN, out=x3[:, p], in0=x3[:, p], in1=w3[:, p]
                        )
                    # E cols: [0]=0, [1..G]=p0 ends, [G+1]=0, [G+2..2G+1]=p1 ends
                    E = small_pool.tile([PP, 2 * V4_G + 2], f32, tag="E")
                    nc.gpsimd.memset(E[:, 0:1], 0.0)
                    nc.gpsimd.memset(E[:, V4_G + 1 : V4_G + 2], 0.0)
                    eins = AP(
                        xt.tensor,
                        xt.offset + 2 * L - 2,
                        [list(xt.ap[0]), [1, 2], [2 * L, V4_G]],
                    )
                    eout = AP(
                        E.tensor,
                        E.offset + 1,
                        [list(E.ap[0]), [V4_G + 1, 2], [1, V4_G]],
                    )
                    nc.scalar.activation(
                        eout, eins, mybir.ActivationFunctionType.Copy
                    )
                    ehi = AP(
                        E.tensor, E.offset + 1,
                        [list(E.ap[0]), [V4_G + 1, 2], [1, V4_G]],
                    )
                    elo = AP(
                        E.tensor, E.offset,
                        [list(E.ap[0]), [V4_G + 1, 2], [1, V4_G]],
                    )
                    ys = small_pool.tile([PP, 2 * V4_G], f32, tag="ys")
                    ys3 = AP(
                        ys.tensor, ys.offset,
                        [list(ys.ap[0]), [V4_G, 2], [1, V4_G]],
                    )
                    nc.vector.tensor_sub(ys3, ehi, elo)
                    b1b = AP(
                        b1g_sb.tensor,
                        b1g_sb.offset + c * V4_G,
                        [list(b1g_sb.ap[0]), [0, 2], [1, V4_G]],
                    )
                    nc.vector.tensor_add(ys3, ys3, b1b)
                    ysr = small_pool.tile([PP, 2 * V4_G], f32, tag="ysr")
                    nc.scalar.activation(
                        ysr[:, :], ys[:, : 2 * V4_G],
                        mybir.ActivationFunctionType.Relu,
                    )
                    yw = small_pool.tile([PP, 2 * V4_G], f32, tag="yw")
                    w2b = AP(
                        w2g_sb.tensor,
                        w2g_sb.offset + c * V4_G,
                        [list(w2g_sb.ap[0]), [0, 2], [1, V4_G]],
                    )
                    nc.vector.tensor_mul(yw[:, :], ysr[:, :], w2b)
                    yw3 = AP(
                        yw.tensor, yw.offset,
                        [list(yw.ap[0]), [V4_G, 2], [1, V4_G]],
                    )
                    pcol = h * 2 * NCHR + 2 * c
                    nc.vector.reduce_sum(
                        parts[:, pcol : pcol + 2], yw3, axis=mybir.AxisListType.X
                    )
            if not do_compute:
                return
            psum_o = psum_pool.tile(
                [V4_TM, 4 * NCHR], f32, tag=f"po_{rep % 2}", name="psum_o"
            )
            nc.tensor.matmul(
                psum_o[:, :], sum5_sb[:, :], parts[:, :], start=True, stop=True
            )
            pcrit = psum_pool.tile([1, 1], f32, tag=f"pc_{rep % 2}")
            for h in range(2):
                outs = small_pool.tile([V4_TM, 2 * NCHR], f32, tag="outs")
                nc.vector.tensor_add(
                    outs[:, :],
                    psum_o[:, h * 2 * NCHR : (h + 1) * 2 * NCHR],
                    b2rep_sb[:, :],
                )
                outsr = small_pool.tile([V4_TM, 2 * NCHR], f32, tag="outsr")
                nc.scalar.activation(
                    outsr[:, :], outs[:, :], mybir.ActivationFunctionType.Relu
                )
                # actor: sum_c outs*wa (per p), max over p, + ba
                aw = small_pool.tile([V4_TM, 2 * NCHR], f32, tag="aw")
                nc.vector.tensor_mul(aw[:, :], outsr[:, :], warep_sb[:, :])
                av = small_pool.tile([V4_TM, 2], f32, tag="av")
                aw3 = AP(aw.tensor, aw.offset, [list(aw.ap[0]), [1, 2], [2, NCHR]])
                nc.vector.reduce_sum(av[:, :], aw3, axis=mybir.AxisListType.X)
                amax = small_pool.tile([V4_TM, 1], f32, tag="amax")
                nc.vector.tensor_max(amax[:, :], av[:, 0:1], av[:, 1:2])
                actor_h = small_pool.tile([V4_TM, 1], f32, tag="actorh")
                nc.scalar.activation(
                    actor_h[:, :], amax[:, :],
                    mybir.ActivationFunctionType.Identity, bias=barep_sb[:, :],
                )
                nc.sync.dma_start(
                    actor_d.ap()[h * V4_TM : (h + 1) * V4_TM, :], actor_h[:, :]
                )
                # critic partial: sum_{p,c} outs*wc
                cw_ = small_pool.tile([V4_TM, 2 * NCHR], f32, tag="cw")
                nc.vector.tensor_mul(cw_[:, :], outsr[:, :], wcrep_sb[:, :])
                cv = small_pool.tile([V4_TM, 1], f32, tag="cv")
                nc.vector.reduce_sum(
                    cv[:, :], cw_[:, :], axis=mybir.AxisListType.X
                )
                nc.tensor.matmul(
                    pcrit[:, :], ones25_sb[:, :], cv[:, :],
                    start=(h == 0), stop=(h == 1),
                )
            crit_sb = out_pool.tile([1, 1], f32, tag="crit")
            nc.scalar.activation(
                crit_sb[:, :], pcrit[:, :],
                mybir.ActivationFunctionType.Identity,
                bias=bcv_sb[:, :], scale=1.0 / (2 * N),
            )
            nc.sync.dma_start(critic_d.ap(), crit_sb[:, :])

        if reps == 1:
            body(0)
        else:
            assert reps % unroll == 0
            with tc.For_i(0, reps // unroll, 1) as _i:
                for u in range(unroll):
                    body(u)

    nc.compile()
    return nc


# --------------------------------------------------------------------------- #
# Entry point
# --------------------------------------------------------------------------- #
_CACHE = {}


def _get_program(seg_key, seg_ids):
    if seg_key not in _CACHE:
        Ls = _uniform_L(seg_ids)
        if Ls is not None:
            nc = build_program_v4(Ls)
            _CACHE[seg_key] = (("v4", Ls), nc)
        else:
            plan = _build_plan(seg_ids)
            nc = build_program(plan)
            _CACHE[seg_key] = (("v1", plan), nc)
    return _CACHE[seg_key]


def make_in_maps_v4(Ls, x, w1, b1, w2, b2, wa, ba, wc, bc):
    packed = _pack_weights_v4(Ls, w1, b1, w2, b2, wa, ba, wc, bc)
    x = np.asarray(x)
    maps = []
    for i in range(x.shape[0]):
        m = {"x": np.ascontiguousarray(x[i], np.float32)}
        m.update(packed)
        maps.append(m)
    return maps


def make_in_maps(plan, x, w1, b1, w2, b2, wa, ba, wc, bc):
    w1p, b1t, w2t = _pack_weights(plan, np.asarray(w1), np.asarray(b1), np.asarray(w2))
    hw = np.stack([np.asarray(wa), np.asarray(wc)], axis=1).astype(np.float32)
    b2c = np.asarray(b2, np.float32).reshape(NCHR, 1)
    hb = np.array([[float(np.asarray(ba)[0]), float(np.asarray(bc)[0])]], np.float32)
    x = np.asarray(x)
    maps = []
    for i in range(x.shape[0]):
        maps.append(
            {
                "x": np.ascontiguousarray(x[i], np.float32),
                "w1p": w1p,
                "b1t": b1t,
                "w2t": w2t,
                "hw": hw,
                "b2": b2c,
                "hb": hb,
            }
        )
    return maps


def kernel(x, w1, b1, w2, b2, wa, ba, wc, bc, seg_ids):
    from concourse import bass_utils

    seg_np = np.asarray(seg_ids)
    seg_key = hash(seg_np.tobytes())
    (kind, meta), nc = _get_program(seg_key, seg_np)
    if kind == "v4":
        in_maps = make_in_maps_v4(meta, x, w1, b1, w2, b2, wa, ba, wc, bc)
    else:
        in_maps = make_in_maps(meta, x, w1, b1, w2, b2, wa, ba, wc, bc)
    res = bass_utils.run_bass_kernel_spmd(nc, in_maps, core_ids=list(range(B)))
    actor = np.stack(
        [np.asarray(res.results[i]["actor"]).reshape(N) for i in range(B)]
    ).astype(np.float32)
    critic = np.array(
        [float(np.asarray(res.results[i]["critic"]).reshape(())) for i in range(B)],
        np.float32,
    )
    return actor, critic
